# revision 10
# baseline (speedup 1.0000x reference)
"""Trainium2 Bass kernel for nn_ConvTemporalGraphical (gnn_message_passing).

Reference computation (see problem):
    A_full[o,k,v,w] = A[k,v,w] * importance[o,k,v,w]          (O,K,V,V)
    y[n,o,t,w]      = sum_{k,v} x[n,0,t,v] * A_full[o,k,v,w]  (N,O,T,V)
    returns (y, A_full)

Only channel 0 of x is used. The k-sum factors out of the x contraction:
    y[n,o,t,w] = sum_v x[n,0,t,v] * B[o,v,w],  B = sum_k A_full[o,k,:,:]
so the device work is a skinny matmul: (N*T, 25) @ (25, 200) per batch shard.

Sharding: data-parallel over batch N across 8 cores (8 batches/core ->
4096 tokens/core). B / importance are tiny and replicated.

Device layout per core:
  - x^T is packed on host into 3 row-groups at partition bases 0/32/64
    (the only legal engine AP bases), 11/11/10 token-tiles per group:
    xp[32*j + v, t'] = x^T[v, tok_off_j + t'].  DMAs are 96 partitions
    wide with multi-KB contiguous runs.
  - B is padded to (25, 256) free (moving free dim 256 => float32r
    streams 1 row/cycle) and replicated at partition bases 0/32/64.
  - 32 matmuls: lhsT = xp[32j:32j+25, 128-token slice] (stationary),
    rhs = bpad[32j:32j+25, :256] (moving), out = PSUM (128, 256) fp32.
  - PSUM -> SBUF copies (DVE/ACT split), staged 4 token-tiles per ybuf,
    8 output DMAs of (128, 4, 200).
  - A_full (an output) is computed on device as impT * at8 in
    [v, (o,k,w)] layout and DMA'd out; host restores the (O,K,V,V) order.
"""

import numpy as np

N, C, T, V = 64, 64, 512, 25
O, K = 8, 3
NCORES = 8
NLOC = N // NCORES          # 8 batches per core
TOK = NLOC * T              # 4096 tokens per core
OW = O * V                  # 200
NTILE = TOK // 128          # 32 token tiles

# 3 row-groups at partition bases 0/32/64: (ntiles) per group
GRP_NTILES = [11, 11, 10]
GRP_TOFF = [0, 11, 22]      # first global tile of each group
XCOLS = 128 * max(GRP_NTILES)      # 1408
XCHUNK_COLS = [512, 512, 384]      # column chunks (128-aligned)

# fp32 matmul is exact (4 cycles/moving-row); float32r is ~TF32 precision
# (~3e-4 rel err on this problem) but streams 1 cycle/row at N>=256.
USE_F32R = False
NPAD = 256 if USE_F32R else OW

_CACHE = {}


def _tile_to_group(gt):
    for j in range(3):
        if gt < GRP_TOFF[j] + GRP_NTILES[j]:
            return j, gt - GRP_TOFF[j]
    raise ValueError(gt)


def _build_nc():
    import concourse.bacc as bacc
    import concourse.mybir as mybir
    import concourse.tile as tile

    f32 = mybir.dt.float32
    mmdt = mybir.dt.float32r if USE_F32R else f32

    nc = bacc.Bacc("TRN2", target_bir_lowering=False, debug=False,
                   enable_asserts=False)

    xp_d = nc.dram_tensor("xp", [96, XCOLS], mmdt, kind="ExternalInput")
    bpad_d = nc.dram_tensor("bpad", [96, NPAD], mmdt, kind="ExternalInput")
    impt_d = nc.dram_tensor("impt", [V, O * K * V], f32, kind="ExternalInput")
    at8_d = nc.dram_tensor("at8", [V, O * K * V], f32, kind="ExternalInput")
    y_d = nc.dram_tensor("y", [NTILE, 128, OW], f32, kind="ExternalOutput")
    afullt_d = nc.dram_tensor("afullt", [V, O * K * V], f32, kind="ExternalOutput")

    with tile.TileContext(nc) as tc:
        with (
            tc.tile_pool(name="const", bufs=1) as cpool,
            tc.tile_pool(name="ybuf", bufs=2) as ypool,
            tc.tile_pool(name="psum", bufs=6, space="PSUM") as pspool,
        ):
            # B weights, padded+replicated (host-prepped): one DMA.
            bp = cpool.tile([96, NPAD], mmdt)
            nc.sync.dma_start(bp[:], bpad_d[:])

            # x^T in 3 column chunks so matmuls start after ~200KB.
            xch = []
            cb = 0
            for cidx, cw in enumerate(XCHUNK_COLS):
                t = cpool.tile([96, cw], mmdt, tag=f"xch{cidx}")
                nc.sync.dma_start(t[:], xp_d[:, cb:cb + cw])
                xch.append((cb, t))
                cb += cw

            # A_full output: prod[v, (o,k,w)] = impT * at8 (off critical path).
            im = cpool.tile([V, O * K * V], f32)
            nc.sync.dma_start(im[:], impt_d[:])
            a8 = cpool.tile([V, O * K * V], f32)
            nc.sync.dma_start(a8[:], at8_d[:])
            pr = cpool.tile([V, O * K * V], f32)
            nc.vector.tensor_mul(pr[:], im[:], a8[:])
            nc.sync.dma_start(afullt_d[:], pr[:])

            # Main loop, round-robin across the 3 row-groups so consecutive
            # matmuls land on independent 32-row PE tiles (can overlap).
            # Per-group ybuf staging of 4 local tiles -> 9 output DMAs.
            order = [(j, lt) for lt in range(max(GRP_NTILES))
                     for j in range(3) if lt < GRP_NTILES[j]]
            ybt = {}
            cnt = 0
            for (j, lt) in order:
                if lt % 4 == 0:
                    ybt[j] = ypool.tile([128, 4, OW], f32, name=f"yb{j}", tag=f"yb{j}")
                col = 128 * lt
                chunk = min(col // 512, 2)
                cb, xt = xch[chunk]
                ps = pspool.tile([128, NPAD], f32, tag="ps")
                lhsT = xt[32 * j:32 * j + V, col - cb:col - cb + 128]
                rhs = bp[32 * j:32 * j + V, :]
                nc.tensor.matmul(ps[:], lhsT, rhs)
                if cnt % 3 < 2:
                    nc.vector.tensor_copy(ybt[j][:, lt % 4, :], ps[:, 0:OW])
                else:
                    nc.scalar.copy(ybt[j][:, lt % 4, :], ps[:, 0:OW])
                cnt += 1
                if lt % 4 == 3 or lt == GRP_NTILES[j] - 1:
                    size = lt % 4 + 1
                    gt0 = GRP_TOFF[j] + (lt // 4) * 4
                    nc.sync.dma_start(
                        y_d[gt0:gt0 + size].rearrange("i p c -> p i c"),
                        ybt[j][:, 0:size, :],
                    )

    nc.compile()
    return nc


def _get_nc():
    if "nc" not in _CACHE:
        _CACHE["nc"] = _build_nc()
    return _CACHE["nc"]


def _host_prep(x, A, importance):
    """Returns in_maps (list of 8 dicts keyed by DRAM tensor name)."""
    x = np.asarray(x, dtype=np.float32)
    A = np.asarray(A, dtype=np.float32)
    importance = np.asarray(importance, dtype=np.float32)

    x0 = np.ascontiguousarray(x[:, 0, :, :])            # (N, T, V)

    a_full = A[None, :, :, :] * importance              # (O, K, V, V)
    B = (a_full[:, 0] + a_full[:, 1]) + a_full[:, 2]    # (O, V, V)
    bmat = B.transpose(1, 0, 2).reshape(V, OW)          # [v, o*V + w]
    bpad = np.zeros((96, NPAD), np.float32)
    for j in range(3):
        bpad[32 * j:32 * j + V, :OW] = bmat

    impt = np.ascontiguousarray(
        importance.transpose(2, 0, 1, 3).reshape(V, O * K * V))
    at8 = np.ascontiguousarray(
        np.tile(A.transpose(1, 0, 2).reshape(V, K * V), (1, O)))

    in_maps = []
    for c in range(NCORES):
        xt = x0[c * NLOC:(c + 1) * NLOC].reshape(TOK, V).T   # (V, TOK)
        xpk = np.zeros((96, XCOLS), np.float32)
        for j in range(3):
            ncols = 128 * GRP_NTILES[j]
            t0 = 128 * GRP_TOFF[j]
            xpk[32 * j:32 * j + V, :ncols] = xt[:, t0:t0 + ncols]
        in_maps.append({
            "xp": xpk,
            "bpad": bpad,
            "impt": impt,
            "at8": at8,
        })
    return in_maps


def _gather(results):
    y = np.empty((N, O, T, V), np.float32)
    for c in range(NCORES):
        yc = np.asarray(results[c]["y"]).reshape(NLOC, T, O, V)
        y[c * NLOC:(c + 1) * NLOC] = yc.transpose(0, 2, 1, 3)
    aft = np.asarray(results[0]["afullt"])               # (V, O*K*V)
    a_full = np.ascontiguousarray(
        aft.reshape(V, O, K, V).transpose(1, 2, 0, 3))   # (O, K, V, V)
    return y, a_full


def kernel(x, A, importance):
    from concourse.bass_utils import run_bass_kernel_spmd

    nc = _get_nc()
    in_maps = _host_prep(x, A, importance)
    res = run_bass_kernel_spmd(nc, in_maps, core_ids=list(range(NCORES)))
    _CACHE["last_results"] = res
    return _gather(res.results)
